# revision 2
# baseline (speedup 1.0000x reference)
"""Trilinear grid_pull on 8 Trainium2 cores (bf16 table variant).

Core c handles batch b=c//4 and output x-slab xq=c%4 (32 planes).
Phase 1 packs input[b] into T[v][8] bf16 (v=(x*128+y)*128+z, slot
k=dx*4+dy*2+c holds input[c, x+dx, y+dy, z]); one 16-value gather at row v
covers rows v and v+1 = all 16 trilinear taps. Gathers run one z-column per
instruction (128 indices, one per partition — the HW limit for
indirect_dma_start). Weights/indices are computed group-wide (8 planes per
DVE op) to cut instruction dispatch.
"""
import numpy as np
from contextlib import ExitStack

from concourse import bass, bacc, mybir
import concourse.tile as tile
from concourse.bass_utils import run_bass_kernel_spmd

P = 128
N = 128
C = 2
B = 2
XS = 32
V = N * N * N
F32 = mybir.dt.float32
I32 = mybir.dt.int32
BF16 = mybir.dt.bfloat16

_CACHE = {}
TRACE = False


def build_kernel(reps=1):
    nc = bacc.Bacc('TRN2', target_bir_lowering=False, num_devices=8)
    inp = nc.dram_tensor("inp", [C, N, N, N], F32, kind="ExternalInput")
    g3 = nc.dram_tensor("g3", [3, XS, N, N], F32, kind="ExternalInput")
    out = nc.dram_tensor("o", [C, XS, N, N], F32, kind="ExternalOutput")
    T = nc.dram_tensor("T", [V, 8], BF16)

    inp_f = inp[:]
    T_f = T[:]

    XG = 8

    with tile.TileContext(nc) as tc:
     for _rep in range(reps):
        with ExitStack() as ctx:
            # ---------------- Phase 1: build T ----------------
            lp = ctx.enter_context(tc.tile_pool(name="load", bufs=3))
            gp = ctx.enter_context(tc.tile_pool(name="gx", bufs=3))

            def load_group(xg):
                ts = {}
                for c in range(C):
                    for sh in range(2):
                        t = lp.tile([P, XG * N], F32, tag=f"ld{c}{sh}")
                        ny = P - sh
                        src = inp_f[c, xg * XG:(xg + 1) * XG, sh:sh + ny, :]
                        src = src.transpose([1, 0, 2])
                        nc.sync.dma_start(
                            t[:ny, :].rearrange("p (a b) -> p a b", a=XG), src)
                        ts[(c, sh)] = t
                return ts

            ngroups = N // XG
            prev = load_group(0)
            for xg in range(ngroups):
                nxt = load_group(xg + 1) if xg + 1 < ngroups else None
                for xo in range(XG):
                    x = xg * XG + xo
                    if x == 127:
                        continue
                    gx_t = gp.tile([P, N, 8], BF16, tag="gx")
                    for dx in range(2):
                        if xo + dx < XG:
                            st, col = prev, xo + dx
                        else:
                            st, col = nxt, 0
                        for dy in range(2):
                            for c in range(C):
                                k = dx * 4 + dy * 2 + c
                                src = st[(c, dy)][:, col * N:(col + 1) * N]
                                nc.vector.tensor_copy(gx_t[:, :, k], src)
                    nc.sync.dma_start(
                        T_f[x * N * N:(x + 1) * N * N, :], gx_t[:, :, :])
                prev = nxt

            # ---------------- Phase 2: gather + lerp ----------------
            pp = ctx.enter_context(tc.tile_pool(name="plane", bufs=2))
            wp = ctx.enter_context(tc.tile_pool(name="wts", bufs=2))
            bp = ctx.enter_context(tc.tile_pool(name="gath", bufs=3))
            op = ctx.enter_context(tc.tile_pool(name="outp", bufs=2))

            XOG = 8
            M = XOG * N  # group-wide free size
            for xog in range(XS // XOG):
                gt = {}
                for d in range(3):
                    t = pp.tile([P, XOG, N], F32, tag=f"g{d}")
                    src = g3[d, xog * XOG:(xog + 1) * XOG, :, :].transpose(
                        [1, 0, 2])
                    nc.sync.dma_start(t[:], src)
                    gt[d] = t
                oc = {}
                for c in range(C):
                    oc_t = op.tile([P, XOG, N], F32, tag=f"oc{c}")
                    oc[c] = oc_t

                for xo in range(XOG):
                    cc = {d: gt[d][:, xo, :] for d in range(3)}
                    ff = {}
                    w = {}
                    for d in range(3):
                        ti = wp.tile([P, N], I32, tag=f"ti{d}")
                        nc.vector.tensor_scalar(ti[:], cc[d], 0.5, None,
                                                mybir.AluOpType.subtract)
                        tfc = wp.tile([P, N], F32, tag=f"tfc{d}")
                        nc.vector.tensor_scalar(tfc[:], ti[:], 0, 126,
                                                mybir.AluOpType.max,
                                                mybir.AluOpType.min)
                        wd = wp.tile([P, N], F32, tag=f"w{d}")
                        nc.vector.tensor_sub(wd[:], cc[d], tfc[:])
                        ff[d] = tfc
                        w[d] = wd
                    t1 = wp.tile([P, N], F32, tag="t1")
                    nc.vector.scalar_tensor_tensor(
                        t1[:], ff[0][:], 128.0, ff[1][:],
                        mybir.AluOpType.mult, mybir.AluOpType.add)
                    idxf = wp.tile([P, N], F32, tag="idxf")
                    nc.vector.scalar_tensor_tensor(
                        idxf[:], t1[:], 128.0, ff[2][:],
                        mybir.AluOpType.mult, mybir.AluOpType.add)
                    idx = wp.tile([P, N], I32, tag="idx")
                    nc.vector.tensor_copy(idx[:], idxf[:])
                    base = 0
                    NQ = 4
                    ZQ = N // NQ
                    for q in range(NQ):
                        gb = bp.tile([P, ZQ, 16], BF16, tag=f"gb{q}")
                        z0 = q * ZQ
                        for zz in range(ZQ):
                            z = z0 + zz
                            nc.gpsimd.indirect_dma_start(
                                out=gb[:, zz, :],
                                out_offset=None,
                                in_=T_f,
                                in_offset=bass.IndirectOffsetOnAxis(
                                    ap=idx[:, z:z + 1], axis=0),
                            )

                        def bc(ap, reps_):
                            return ap.unsqueeze(2).broadcast_to([P, ZQ, reps_])

                        wzq = w[2][:, z0:z0 + ZQ]
                        wxq = w[0][:, z0:z0 + ZQ]
                        wyq = w[1][:, z0:z0 + ZQ]
                        vz = bp.tile([P, ZQ, 8], F32, tag=f"vz{q}")
                        nc.vector.tensor_sub(vz[:], gb[:, :, 8:16],
                                             gb[:, :, 0:8])
                        nc.vector.tensor_mul(vz[:], vz[:], bc(wzq, 8))
                        nc.vector.tensor_add(vz[:], vz[:], gb[:, :, 0:8])

                        vx = bp.tile([P, ZQ, 4], F32, tag=f"vx{q}")
                        nc.vector.tensor_sub(vx[:], vz[:, :, 4:8],
                                             vz[:, :, 0:4])
                        nc.vector.tensor_mul(vx[:], vx[:], bc(wxq, 4))
                        nc.vector.tensor_add(vx[:], vx[:], vz[:, :, 0:4])

                        vy = bp.tile([P, ZQ, 2], F32, tag=f"vy{q}")
                        nc.vector.tensor_sub(vy[:], vx[:, :, 2:4],
                                             vx[:, :, 0:2])
                        nc.vector.tensor_mul(vy[:], vy[:], bc(wyq, 2))
                        nc.vector.tensor_add(vy[:], vy[:], vx[:, :, 0:2])

                        for c in range(C):
                            nc.vector.tensor_copy(oc[c][:, xo, z0:z0 + ZQ],
                                                  vy[:, :, c])

                for c in range(C):
                    dst = out[:][c, xog * XOG:(xog + 1) * XOG, :, :].transpose(
                        [1, 0, 2])
                    nc.sync.dma_start(dst, oc[c][:])

    nc.compile()
    return nc


def make_in_maps(input, grid):
    in_maps = []
    for core in range(8):
        b, xq = core // 4, core % 4
        in_maps.append({
            "inp": input[b],
            "g3": np.ascontiguousarray(grid[b, :, xq * XS:(xq + 1) * XS]),
        })
    return in_maps


def kernel(input, grid):
    input = np.ascontiguousarray(input, dtype=np.float32)
    grid = np.ascontiguousarray(grid, dtype=np.float32)
    key = "nc"
    if key not in _CACHE:
        _CACHE[key] = build_kernel()
    nc = _CACHE[key]
    in_maps = make_in_maps(input, grid)
    res = run_bass_kernel_spmd(nc, in_maps, core_ids=list(range(8)),
                               trace=TRACE)
    out = np.empty((B, C, N, N, N), dtype=np.float32)
    for core in range(8):
        b, xq = core // 4, core % 4
        out[b, :, xq * XS:(xq + 1) * XS] = res.results[core]["o"]
    return out


if __name__ == "__main__":
    rng = np.random.default_rng(0)
    inp = rng.standard_normal((B, C, N, N, N)).astype(np.float32)
    grid = (rng.random((B, 3, N, N, N), dtype=np.float32) * (N - 1)).astype(
        np.float32)
    got = kernel(inp, grid)
    print(got.shape, got.dtype)


# revision 3
# speedup vs baseline: 10.3982x; 10.3982x over previous
"""Trilinear grid_pull on 8 Trainium2 cores (bf16 table variant).

Core c handles batch b=c//4 and output x-slab xq=c%4 (32 planes).
Phase 1 packs input[b] into T[v][8] bf16 (v=(x*128+y)*128+z, slot
k=dx*4+dy*2+c holds input[c, x+dx, y+dy, z]); one 16-value gather at row v
covers rows v and v+1 = all 16 trilinear taps. Gathers run one z-column per
instruction (128 indices, one per partition — the HW limit for
indirect_dma_start). Weights/indices are computed group-wide (8 planes per
DVE op) to cut instruction dispatch.
"""
import numpy as np
from contextlib import ExitStack

from concourse import bass, bacc, mybir
import concourse.tile as tile
from concourse.bass_utils import run_bass_kernel_spmd

P = 128
N = 128
C = 2
B = 2
XS = 32
V = N * N * N
F32 = mybir.dt.float32
I32 = mybir.dt.int32
BF16 = mybir.dt.bfloat16

_CACHE = {}
TRACE = False


def build_kernel(reps=1):
    nc = bacc.Bacc('TRN2', target_bir_lowering=False, num_devices=8)
    inp = nc.dram_tensor("inp", [C, N, N, N], F32, kind="ExternalInput")
    g3 = nc.dram_tensor("g3", [3, XS, N, N], F32, kind="ExternalInput")
    out = nc.dram_tensor("o", [C, XS, N, N], F32, kind="ExternalOutput")
    T = nc.dram_tensor("T", [V, 8], BF16)

    inp_f = inp[:]
    T_f = T[:]

    XG = 8

    with tile.TileContext(nc) as tc:
     for _rep in range(reps):
        with ExitStack() as ctx:
            # ---------------- Phase 1: build T ----------------
            lp = ctx.enter_context(tc.tile_pool(name="load", bufs=3))
            gp = ctx.enter_context(tc.tile_pool(name="gx", bufs=3))

            def load_group(xg):
                ts = {}
                for c in range(C):
                    for sh in range(2):
                        t = lp.tile([P, XG * N], F32, tag=f"ld{c}{sh}")
                        ny = P - sh
                        src = inp_f[c, xg * XG:(xg + 1) * XG, sh:sh + ny, :]
                        src = src.transpose([1, 0, 2])
                        nc.sync.dma_start(
                            t[:ny, :].rearrange("p (a b) -> p a b", a=XG), src)
                        ts[(c, sh)] = t
                return ts

            ngroups = N // XG
            prev = load_group(0)
            for xg in range(ngroups):
                nxt = load_group(xg + 1) if xg + 1 < ngroups else None
                for xo in range(XG):
                    x = xg * XG + xo
                    if x == 127:
                        continue
                    gx_t = gp.tile([P, N, 8], BF16, tag="gx")
                    for dx in range(2):
                        if xo + dx < XG:
                            st, col = prev, xo + dx
                        else:
                            st, col = nxt, 0
                        for dy in range(2):
                            for c in range(C):
                                k = dx * 4 + dy * 2 + c
                                src = st[(c, dy)][:, col * N:(col + 1) * N]
                                nc.vector.tensor_copy(gx_t[:, :, k], src)
                    nc.sync.dma_start(
                        T_f[x * N * N:(x + 1) * N * N, :], gx_t[:, :, :])
                prev = nxt

            # ---------------- Phase 2: gather + lerp ----------------
            pp = ctx.enter_context(tc.tile_pool(name="plane", bufs=2))
            wp = ctx.enter_context(tc.tile_pool(name="wts", bufs=2))
            bp = ctx.enter_context(tc.tile_pool(name="gath", bufs=3))
            gbp = ctx.enter_context(tc.tile_pool(name="gbuf", bufs=8))
            op = ctx.enter_context(tc.tile_pool(name="outp", bufs=2))

            XOG = 8
            M = XOG * N  # group-wide free size
            for xog in range(XS // XOG):
                gt = {}
                for d in range(3):
                    t = pp.tile([P, XOG, N], F32, tag=f"g{d}")
                    src = g3[d, xog * XOG:(xog + 1) * XOG, :, :].transpose(
                        [1, 0, 2])
                    nc.sync.dma_start(t[:], src)
                    gt[d] = t
                oc = {}
                for c in range(C):
                    oc_t = op.tile([P, XOG, N], F32, tag=f"oc{c}")
                    oc[c] = oc_t

                for xo in range(XOG):
                    cc = {d: gt[d][:, xo, :] for d in range(3)}
                    ff = {}
                    w = {}
                    for d in range(3):
                        ti = wp.tile([P, N], I32, tag=f"ti{d}")
                        nc.vector.tensor_scalar(ti[:], cc[d], 0.5, None,
                                                mybir.AluOpType.subtract)
                        tfc = wp.tile([P, N], F32, tag=f"tfc{d}")
                        nc.vector.tensor_scalar(tfc[:], ti[:], 0, 126,
                                                mybir.AluOpType.max,
                                                mybir.AluOpType.min)
                        wd = wp.tile([P, N], F32, tag=f"w{d}")
                        nc.vector.tensor_sub(wd[:], cc[d], tfc[:])
                        ff[d] = tfc
                        w[d] = wd
                    t1 = wp.tile([P, N], F32, tag="t1")
                    nc.vector.scalar_tensor_tensor(
                        t1[:], ff[0][:], 128.0, ff[1][:],
                        mybir.AluOpType.mult, mybir.AluOpType.add)
                    idxf = wp.tile([P, N], F32, tag="idxf")
                    nc.vector.scalar_tensor_tensor(
                        idxf[:], t1[:], 128.0, ff[2][:],
                        mybir.AluOpType.mult, mybir.AluOpType.add)
                    idx = wp.tile([P, N], I32, tag="idx")
                    nc.vector.tensor_copy(idx[:], idxf[:])
                    base = 0
                    NQ = 4
                    ZQ = N // NQ
                    for q in range(NQ):
                        gb = gbp.tile([P, ZQ, 16], BF16, tag=f"gb{q}")
                        z0 = q * ZQ
                        for zz in range(ZQ):
                            z = z0 + zz
                            nc.gpsimd.indirect_dma_start(
                                out=gb[:, zz, :],
                                out_offset=None,
                                in_=T_f,
                                in_offset=bass.IndirectOffsetOnAxis(
                                    ap=idx[:, z:z + 1], axis=0),
                            )

                        def bc(ap, reps_):
                            return ap.unsqueeze(2).broadcast_to([P, ZQ, reps_])

                        wzq = w[2][:, z0:z0 + ZQ]
                        wxq = w[0][:, z0:z0 + ZQ]
                        wyq = w[1][:, z0:z0 + ZQ]
                        vz = bp.tile([P, ZQ, 8], F32, tag=f"vz{q}")
                        nc.vector.tensor_sub(vz[:], gb[:, :, 8:16],
                                             gb[:, :, 0:8])
                        nc.vector.tensor_mul(vz[:], vz[:], bc(wzq, 8))
                        nc.vector.tensor_add(vz[:], vz[:], gb[:, :, 0:8])

                        vx = bp.tile([P, ZQ, 4], F32, tag=f"vx{q}")
                        nc.vector.tensor_sub(vx[:], vz[:, :, 4:8],
                                             vz[:, :, 0:4])
                        nc.vector.tensor_mul(vx[:], vx[:], bc(wxq, 4))
                        nc.vector.tensor_add(vx[:], vx[:], vz[:, :, 0:4])

                        vy = bp.tile([P, ZQ, 2], F32, tag=f"vy{q}")
                        nc.vector.tensor_sub(vy[:], vx[:, :, 2:4],
                                             vx[:, :, 0:2])
                        nc.vector.tensor_mul(vy[:], vy[:], bc(wyq, 2))
                        nc.vector.tensor_add(vy[:], vy[:], vx[:, :, 0:2])

                        for c in range(C):
                            nc.vector.tensor_copy(oc[c][:, xo, z0:z0 + ZQ],
                                                  vy[:, :, c])

                for c in range(C):
                    dst = out[:][c, xog * XOG:(xog + 1) * XOG, :, :].transpose(
                        [1, 0, 2])
                    nc.sync.dma_start(dst, oc[c][:])

    nc.compile()
    return nc


def make_in_maps(input, grid):
    in_maps = []
    for core in range(8):
        b, xq = core // 4, core % 4
        in_maps.append({
            "inp": input[b],
            "g3": np.ascontiguousarray(grid[b, :, xq * XS:(xq + 1) * XS]),
        })
    return in_maps


def kernel(input, grid):
    input = np.ascontiguousarray(input, dtype=np.float32)
    grid = np.ascontiguousarray(grid, dtype=np.float32)
    key = "nc"
    if key not in _CACHE:
        _CACHE[key] = build_kernel()
    nc = _CACHE[key]
    in_maps = make_in_maps(input, grid)
    res = run_bass_kernel_spmd(nc, in_maps, core_ids=list(range(8)),
                               trace=TRACE)
    out = np.empty((B, C, N, N, N), dtype=np.float32)
    for core in range(8):
        b, xq = core // 4, core % 4
        out[b, :, xq * XS:(xq + 1) * XS] = res.results[core]["o"]
    return out


if __name__ == "__main__":
    rng = np.random.default_rng(0)
    inp = rng.standard_normal((B, C, N, N, N)).astype(np.float32)
    grid = (rng.random((B, 3, N, N, N), dtype=np.float32) * (N - 1)).astype(
        np.float32)
    got = kernel(inp, grid)
    print(got.shape, got.dtype)
